# revision 9
# baseline (speedup 1.0000x reference)
"""Trainium2 Bass kernel for nn_ConformerMHA (LN -> QKV+RoPE -> MHA -> out-proj).

Sharding: pure data-parallel over batch (B=8 -> 8 cores), weights replicated.
No collectives needed.

Per-core dataflow (one batch b, T=2048, D=512, H=8, dk=64):
  A. LayerNorm over x[b] in [T,D] tiles, then PE-transpose -> hT [D, T] (f32).
     ln_w/ln_b are folded into the QKV weights/biases host-side.
  B. QKV projections from hT:
       Q^T, K^T (and their rotate-half copies via sign-permuted weight copies)
       in [feat, T] layout; RoPE applied as qhat = (Q+bq) .* cos + (Qrot+brot) .* sin
       with cos/sin tables [128, T] (pattern repeats per 64-feature head, x2 heads
       per 128 partitions).  Output qhat/khat bf16 [128, pair, T].
       V in [T, feat] layout -> V' bf16 [128part=T-tile, kt, head, 65] where col 64
       holds (1 - mask) and all 64 V columns are scaled by (1 - mask): this
       implements masked softmax exactly (masked keys contribute 0 to both the
       numerator and the denominator).
  C. Attention per (head, q-chunk of 512):
       S^T chunks = khat_slice.T @ qhat_slice -> PSUM [128 keys, 512 q]
       exp via ScalarE activation (scale=1/sqrt(dk) folded in), bf16 out
       A@V: out^T[65, 512] = sum_kt V'[kt,h].T @ expS[kt]  (row 64 = denominator)
       normalize: attn = num * (1/den) broadcast via DMA.
  D. out-proj: out[t,:] = attnT.T @ Wo (+ b_o), DMA to HBM.

Matmuls run as float32r (full PE rate at free-dim>=256) except the attention
inner matmuls which run bf16 (inputs quantized anyway by design).
"""

import numpy as np

B, T, D = 8, 2048, 512
H, DK = 8, 64
P = 128
TT = T // P          # 16 key/row tiles
KC = D // P          # 4 contraction chunks of the model dim
QC = 4               # number of query chunks
QW = T // QC         # 512 query positions per chunk
EPS = 1e-5
SCALE = 1.0 / np.sqrt(np.float32(DK))

# kc groups for the S^T psum staging: (start_tile, n_tiles). Sizes chosen so
# two live groups (4+2 banks) + A@V accumulators (2 banks) fit in 8 PSUM banks.
S_GROUPS = ((0, 4), (4, 2), (6, 4), (10, 2), (12, 4))


def _host_prep(inputs):
    x = np.ascontiguousarray(np.asarray(inputs["x"], dtype=np.float32))
    mask = np.asarray(inputs["mask"]).astype(bool)
    ln_w = np.asarray(inputs["ln_w"], dtype=np.float32)
    ln_b = np.asarray(inputs["ln_b"], dtype=np.float32)
    w_qkv = np.asarray(inputs["w_qkv"], dtype=np.float32)
    b_qkv = np.asarray(inputs["b_qkv"], dtype=np.float32)
    w_o = np.ascontiguousarray(np.asarray(inputs["w_o"], dtype=np.float32))
    b_o = np.asarray(inputs["b_o"], dtype=np.float32)

    # Fold LN affine into the QKV projection:
    #   (h*ln_w + ln_b) @ W + b  ==  h @ (ln_w[:,None]*W) + (ln_b@W + b)
    w_fold = ln_w[:, None] * w_qkv                      # (512, 1536)
    b_fold = ln_b @ w_qkv + b_qkv                       # (1536,)
    Wq, Wk, Wv = w_fold[:, :D], w_fold[:, D:2 * D], w_fold[:, 2 * D:]
    bq, bk, bv = b_fold[:D], b_fold[D:2 * D], b_fold[2 * D:]

    # rotate-half permutation with signs, applied per 64-wide head
    j = np.arange(D)
    loc = j % DK
    src = np.where(loc < DK // 2, j + DK // 2, j - DK // 2)
    sgn = np.where(loc < DK // 2, -1.0, 1.0).astype(np.float32)
    Wqrot = Wq[:, src] * sgn
    Wkrot = Wk[:, src] * sgn
    bqrot = bq[src] * sgn
    bkrot = bk[src] * sgn

    wext = np.ascontiguousarray(
        np.concatenate([Wq, Wk, Wqrot, Wkrot, Wv], axis=1))  # (512, 2560)

    # per-partition bias scalars for the transposed Q/K(+rot) chunks:
    # column layout: [Q0..Q3, K0..K3, Qrot0..3, Krot0..3]
    bqk = np.zeros((P, 16), dtype=np.float32)
    for r, bvec in enumerate((bq, bk, bqrot, bkrot)):
        for fc in range(KC):
            bqk[:, r * KC + fc] = bvec[fc * P:(fc + 1) * P]

    # rope tables, [128, T]: row p uses inv_freq[p % 32]
    inv_freq = (1.0 / (10000.0 ** (np.arange(0, DK, 2, dtype=np.float32) / DK)))
    ang = np.arange(T, dtype=np.float32)[:, None] * inv_freq[None, :]  # (T, 32)
    cost = np.ascontiguousarray(np.tile(np.cos(ang).T, (4, 1)).astype(np.float32))
    sint = np.ascontiguousarray(np.tile(np.sin(ang).T, (4, 1)).astype(np.float32))

    # (1 - mask) in [128, TT] per-partition layout per batch
    maskf = np.zeros((B, P, TT), dtype=np.float32)
    for b in range(B):
        maskf[b] = (1.0 - mask[b].astype(np.float32)).reshape(TT, P).T

    return dict(x=x, wext=wext, bqk=bqk, cost=cost, sint=sint, maskf=maskf,
                wo=w_o, bv=bv.astype(np.float32), bo=b_o.astype(np.float32))


def _build_bass(has_bv, has_bo, debug=False):
    import concourse.bass as bass
    import concourse.mybir as mybir
    import concourse.tile as tile
    from concourse import bacc
    from concourse.masks import make_identity

    F32 = mybir.dt.float32
    F32R = mybir.dt.float32r
    BF16 = mybir.dt.bfloat16
    AF = mybir.ActivationFunctionType
    OP = mybir.AluOpType

    nc = bacc.Bacc()
    xb = nc.dram_tensor("xb", [T, D], F32, kind="ExternalInput")
    maskf_d = nc.dram_tensor("maskf", [P, TT], F32, kind="ExternalInput")
    wext_d = nc.dram_tensor("wext", [D, 5 * D], F32R, kind="ExternalInput")
    bqk_d = nc.dram_tensor("bqk", [P, 16], F32, kind="ExternalInput")
    cost_d = nc.dram_tensor("cost", [P, T], F32, kind="ExternalInput")
    sint_d = nc.dram_tensor("sint", [P, T], F32, kind="ExternalInput")
    wo_d = nc.dram_tensor("wo", [D, D], F32R, kind="ExternalInput")
    if has_bv:
        bv_d = nc.dram_tensor("bv", [D], F32, kind="ExternalInput")
    if has_bo:
        bo_d = nc.dram_tensor("bo", [D], F32, kind="ExternalInput")
    out_d = nc.dram_tensor("out", [T, D], F32, kind="ExternalOutput")
    if debug:
        dbg_ht = nc.dram_tensor("dbg_ht", [P, KC, T], F32R, kind="ExternalOutput")
        dbg_q = nc.dram_tensor("dbg_q", [P, KC, T], BF16, kind="ExternalOutput")
        dbg_k = nc.dram_tensor("dbg_k", [P, KC, T], BF16, kind="ExternalOutput")
        dbg_v = nc.dram_tensor("dbg_v", [P, TT, H, DK + 1], BF16, kind="ExternalOutput")
        dbg_a = nc.dram_tensor("dbg_a", [P, KC, T], F32R, kind="ExternalOutput")

    def r32(ap):
        return ap.bitcast(F32R)

    with tile.TileContext(nc) as tc:
        with tc.tile_pool(name="consts", bufs=1) as consts, \
             tc.tile_pool(name="persist", bufs=1) as persist:
            # ---- constants ----
            ident = consts.tile([P, P], F32)
            make_identity(nc, ident)
            eps_t = consts.tile([P, 1], F32)
            nc.vector.memset(eps_t, EPS)
            cos_s = consts.tile([P, T], F32)
            nc.sync.dma_start(out=cos_s, in_=cost_d[:, :])
            sin_s = consts.tile([P, T], F32)
            nc.sync.dma_start(out=sin_s, in_=sint_d[:, :])
            maskf_s = consts.tile([P, TT], F32)
            nc.sync.dma_start(out=maskf_s, in_=maskf_d[:, :])
            bqk_s = consts.tile([P, 16], F32)
            nc.sync.dma_start(out=bqk_s, in_=bqk_d[:, :])
            wv_s = consts.tile([P, KC, D], F32R)
            nc.sync.dma_start(
                out=wv_s,
                in_=wext_d[:, 4 * D:5 * D].rearrange("(kc p) f -> p kc f", p=P))
            wo_s = consts.tile([P, KC, D], F32R)
            nc.sync.dma_start(
                out=wo_s, in_=wo_d[:, :].rearrange("(kc p) f -> p kc f", p=P))
            if has_bv:
                bv_s = consts.tile([P, D], F32)
                nc.gpsimd.dma_start(out=bv_s, in_=bv_d[:].partition_broadcast(P))
            if has_bo:
                bo_s = consts.tile([P, D], F32)
                nc.gpsimd.dma_start(out=bo_s, in_=bo_d[:].partition_broadcast(P))

            # ---- persistent intermediates ----
            hT = persist.tile([P, KC, T], F32R)         # 32 KB/part
            qhat = persist.tile([P, KC, T], BF16)       # 16 KB/part
            khat = persist.tile([P, KC, T], BF16)       # 16 KB/part
            vp = persist.tile([P, TT, H, DK + 1], BF16)  # ~16.3 KB/part
            attnT = persist.tile([P, KC, T], F32R)      # 32 KB/part

            # ================= Phase A: LayerNorm + transpose =================
            with tc.tile_pool(name="ab_work", bufs=3) as work, \
                 tc.tile_pool(name="ab_psum", bufs=1, space="PSUM") as apsum:
                for ti in range(TT):
                    xt = work.tile([P, D], F32, tag="x")
                    nc.sync.dma_start(out=xt, in_=xb[ti * P:(ti + 1) * P, :])
                    st = work.tile([P, 6], F32, tag="st")
                    nc.vector.bn_stats(out=st, in_=xt)
                    mv = work.tile([P, 2], F32, tag="mv")
                    nc.vector.bn_aggr(out=mv, in_=st)
                    rstd = work.tile([P, 1], F32, tag="rstd")
                    nc.scalar.activation(out=rstd, in_=mv[:, 1:2], func=AF.Sqrt,
                                         bias=eps_t, scale=1.0)
                    nc.vector.reciprocal(out=rstd, in_=rstd)
                    ht = work.tile([P, D], F32, tag="h")
                    nc.vector.tensor_scalar(out=ht, in0=xt,
                                            scalar1=mv[:, 0:1], scalar2=rstd,
                                            op0=OP.subtract, op1=OP.mult)
                    for dc in range(KC):
                        tp = apsum.tile([P, P], F32, tag="tp", bufs=2)
                        nc.tensor.transpose(tp, ht[:, dc * P:(dc + 1) * P], ident)
                        nc.scalar.copy(out=hT[:, dc, ti * P:(ti + 1) * P], in_=tp)

                # ================= Phase B: QKV + RoPE =================
                # Q/K transposed (+rot) -> qhat/khat
                for role in range(2):            # 0 = Q, 1 = K
                    dest = qhat if role == 0 else khat
                    for fc in range(KC):
                        wm = work.tile([P, KC, P], F32R, tag="wm")
                        nc.sync.dma_start(
                            out=wm,
                            in_=wext_d[:, (role * KC + fc) * P:(role * KC + fc + 1) * P]
                            .rearrange("(kc p) f -> p kc f", p=P))
                        wr = work.tile([P, KC, P], F32R, tag="wr")
                        nc.sync.dma_start(
                            out=wr,
                            in_=wext_d[:, ((2 + role) * KC + fc) * P:((2 + role) * KC + fc + 1) * P]
                            .rearrange("(kc p) f -> p kc f", p=P))
                        for qt in range(QC):
                            pm = apsum.tile([P, QW], F32, tag="proj", bufs=4)
                            pr = apsum.tile([P, QW], F32, tag="proj", bufs=4)
                            for kc in range(KC):
                                nc.tensor.matmul(
                                    pm, lhsT=wm[:, kc, :],
                                    rhs=hT[:, kc, qt * QW:(qt + 1) * QW],
                                    start=(kc == 0), stop=(kc == KC - 1))
                            for kc in range(KC):
                                nc.tensor.matmul(
                                    pr, lhsT=wr[:, kc, :],
                                    rhs=hT[:, kc, qt * QW:(qt + 1) * QW],
                                    start=(kc == 0), stop=(kc == KC - 1))
                            t1 = work.tile([P, QW], F32, tag="t1")
                            nc.vector.scalar_tensor_tensor(
                                out=t1, in0=pm,
                                scalar=bqk_s[:, role * KC + fc:role * KC + fc + 1],
                                in1=cos_s[:, qt * QW:(qt + 1) * QW],
                                op0=OP.add, op1=OP.mult)
                            t2 = work.tile([P, QW], F32, tag="t2")
                            nc.vector.scalar_tensor_tensor(
                                out=t2, in0=pr,
                                scalar=bqk_s[:, (2 + role) * KC + fc:(2 + role) * KC + fc + 1],
                                in1=sin_s[:, qt * QW:(qt + 1) * QW],
                                op0=OP.add, op1=OP.mult)
                            nc.vector.tensor_add(
                                out=dest[:, fc, qt * QW:(qt + 1) * QW],
                                in0=t1, in1=t2)

                # V in [T, feat] layout -> masked V' (+ mask column)
                for ti in range(TT):
                    pv = apsum.tile([P, D], F32, tag="proj", bufs=4)
                    for kc in range(KC):
                        nc.tensor.matmul(
                            pv, lhsT=hT[:, kc, ti * P:(ti + 1) * P],
                            rhs=wv_s[:, kc, :],
                            start=(kc == 0), stop=(kc == KC - 1))
                    if has_bv:
                        nc.vector.tensor_add(out=pv, in0=pv, in1=bv_s)
                    nc.vector.tensor_scalar_mul(
                        out=vp[:, ti, :, 0:DK],
                        in0=pv.rearrange("p (h e) -> p h e", h=H),
                        scalar1=maskf_s[:, ti:ti + 1])
                    nc.vector.tensor_copy(
                        out=vp[:, ti, :, DK:DK + 1],
                        in_=maskf_s[:, ti:ti + 1].to_broadcast((P, H, 1)))

            # ================= Phase C: attention =================
            with tc.tile_pool(name="c_work", bufs=1) as cwork, \
                 tc.tile_pool(name="c_dram", bufs=3, space="DRAM") as cdram, \
                 tc.tile_pool(name="c_psum", bufs=1, space="PSUM") as cpsum:
                for h in range(H):
                    pt, ph = h // 2, h % 2
                    qsl = qhat[ph * DK:(ph + 1) * DK, pt, :]
                    ksl = khat[ph * DK:(ph + 1) * DK, pt, :]
                    for qc in range(QC):
                        avp = cpsum.tile([DK + 1, QW], F32, tag="av", bufs=2)
                        for gi, (k0, glen) in enumerate(S_GROUPS):
                            sg = cpsum.tile([P, glen * QW], F32,
                                            tag=f"sg{glen}", bufs=1)
                            eg = cwork.tile([P, glen, QW], BF16,
                                            tag=f"eg{glen}", bufs=3)
                            for jj in range(glen):
                                kt = k0 + jj
                                nc.tensor.matmul(
                                    sg[:, jj * QW:(jj + 1) * QW],
                                    lhsT=ksl[:, kt * P:(kt + 1) * P],
                                    rhs=qsl[:, qc * QW:(qc + 1) * QW],
                                    start=True, stop=True)
                            nc.scalar.activation(
                                out=eg,
                                in_=sg.rearrange("p (g q) -> p g q", g=glen),
                                func=AF.Exp, scale=float(SCALE))
                            for jj in range(glen):
                                kt = k0 + jj
                                nc.tensor.matmul(
                                    avp, lhsT=vp[:, kt, h, :], rhs=eg[:, jj, :],
                                    start=(kt == 0), stop=(kt == TT - 1))
                        # 1/den: psum row -> sbuf row -> [64,8] split ->
                        # exact reciprocal -> DRAM bounce -> [64,512] bcast
                        den_sb = cwork.tile([P, QW], F32, tag="densb", bufs=2)
                        nc.scalar.copy(out=den_sb[DK:DK + 1, :],
                                       in_=avp[DK:DK + 1, :])
                        rec = cwork.tile([DK, QW // DK], F32, tag="rec", bufs=2)
                        nc.sync.dma_start(out=rec, in_=den_sb[DK:DK + 1, :])
                        nc.vector.reciprocal(out=rec, in_=rec)
                        dsc = cdram.tile([QW], F32, tag="dsc")
                        nc.sync.dma_start(out=dsc, in_=rec)
                        invb = cwork.tile([DK, QW], F32, tag="invb", bufs=2)
                        nc.gpsimd.dma_start(
                            out=invb,
                            in_=bass.AP(tensor=dsc.tensor, offset=dsc.offset,
                                        ap=[[0, DK], list(dsc.ap[0])]))
                        stage = cwork.tile([DK, QW], F32R, tag="stage", bufs=2)
                        nc.vector.tensor_mul(out=stage, in0=avp[0:DK, :], in1=invb)
                        nc.sync.dma_start(
                            out=attnT[ph * DK:(ph + 1) * DK, pt,
                                      qc * QW:(qc + 1) * QW],
                            in_=stage)

            # ================= Phase D: output projection =================
            with tc.tile_pool(name="d_work", bufs=3) as dwork, \
                 tc.tile_pool(name="d_psum", bufs=4, space="PSUM") as dpsum:
                for ti in range(TT):
                    po = dpsum.tile([P, D], F32, tag="op")
                    for fc in range(KC):
                        nc.tensor.matmul(
                            po, lhsT=attnT[:, fc, ti * P:(ti + 1) * P],
                            rhs=wo_s[:, fc, :],
                            start=(fc == 0), stop=(fc == KC - 1))
                    ot = dwork.tile([P, D], F32, tag="o")
                    if has_bo:
                        nc.vector.tensor_add(out=ot, in0=po, in1=bo_s)
                    else:
                        nc.scalar.copy(out=ot, in_=po)
                    nc.sync.dma_start(out=out_d[ti * P:(ti + 1) * P, :], in_=ot)

            if debug:
                with tc.tile_pool(name="dbgp", bufs=1) as _dp:
                    nc.sync.dma_start(out=dbg_ht[:, :, :], in_=hT)
                    nc.sync.dma_start(out=dbg_q[:, :, :], in_=qhat)
                    nc.sync.dma_start(out=dbg_k[:, :, :], in_=khat)
                    nc.sync.dma_start(out=dbg_v[:, :, :, :], in_=vp)
                    nc.sync.dma_start(out=dbg_a[:, :, :], in_=attnT)

    nc.compile()
    return nc


_CACHE = {}


def _get_bass(has_bv, has_bo):
    key = (has_bv, has_bo)
    if key not in _CACHE:
        _CACHE[key] = _build_bass(has_bv, has_bo)
    return _CACHE[key]


def kernel(**inputs) -> np.ndarray:
    from concourse.bass_utils import run_bass_kernel_spmd

    prep = _host_prep(inputs)
    has_bv = bool(np.any(prep["bv"]))
    has_bo = bool(np.any(prep["bo"]))
    nc = _get_bass(has_bv, has_bo)

    in_maps = []
    for b in range(B):
        m = dict(xb=np.ascontiguousarray(prep["x"][b]),
                 maskf=np.ascontiguousarray(prep["maskf"][b]),
                 wext=prep["wext"], bqk=prep["bqk"],
                 cost=prep["cost"], sint=prep["sint"], wo=prep["wo"])
        if has_bv:
            m["bv"] = prep["bv"]
        if has_bo:
            m["bo"] = prep["bo"]
        in_maps.append(m)

    res = run_bass_kernel_spmd(nc, in_maps, core_ids=list(range(B)))
    return np.stack([r["out"] for r in res.results]).astype(np.float32)
